# revision 2
# baseline (speedup 1.0000x reference)
"""Trainium2 Bass kernel for nn_GatedJunction (gated multi-branch junction).

Math (per batch element b):
    m_y  = mean_hw(y[b])                     # [C]
    m_xk = mean_hw(x_k[b])                   # [C] for k=0..3
    feats = concat(m_y, m_x0..m_x3)          # [5C] = [1280]
    h  = relu(bn(feats @ conv1_w.T))         # [32]
    w  = h @ conv2_w.T + conv2_b             # [1280] -> [5, 256]
    w1 = sigmoid(w[0])                       # self gate  [256]
    w2 = softmax_k(w[1:])                    # branch gates [4, 256]
    out[b] = y[b]*w1[:,None,None] + sum_k w2[k][:,None,None]*x_k[b]

Sharding: data-parallel over batch. 8 cores x 4 batch elements each.
Params are tiny; they are pre-transposed/folded on the host and replicated.

v2 design notes (vs the v1 baseline):
- Host-side param prep: conv1_w/conv2_w pre-transposed into the on-chip
  layouts, BN folded to (scale_eff, bias_eff), conv2_b folded into an
  augmented 33rd weight row, and the sigmoid columns pre-negated so that
  ALL ten gate columns need only a single Exp activation.  The ACT engine
  therefore runs exactly one function (Exp) -> no activation-table reloads.
- All other elementwise work (means via tensor_scalar accum, BN affine,
  relu via max, sigmoid reconstruction 1/(1+e^-v), softmax normalization,
  pass-2 FMAs) runs on DVE (2x fp32 mode).
- DMA is spread over independent engine queues: loads of y/x1/x2 on the
  SP queue, x0/x3 on the Pool queue, output stores on the ACT queue, so
  transfers pipeline and per-DMA gaps overlap across queues.
"""

import sys

for _p in ("/root/.axon_site/_ro/trn_rl_repo", "/opt/trn_rl_repo"):
    if _p not in sys.path:
        sys.path.append(_p)

from contextlib import ExitStack

import numpy as np

import concourse.bass as bass
import concourse.tile as tile
from concourse import mybir
from concourse.bass_utils import run_bass_kernel_spmd

# Problem constants (hardcoded from the spec).
B, K, C, H, W = 32, 4, 256, 32, 32
MID = 32
EPS = 1e-5
HW = H * W          # 1024
N_CORES = 8
B_LOC = B // N_CORES  # 4
NT = K + 1          # 5 tensors: y, x0..x3
FEAT = NT * C       # 1280
NCH = FEAT // 128   # 10 feature chunks of 128
CH = C // 128       # 2 channel chunks per tensor
MIDA = MID + 1      # 33: h augmented with a constant-1 row (bias fold)

FP32 = mybir.dt.float32
ALU = mybir.AluOpType
AF = mybir.ActivationFunctionType


def _split_waits(nc: bass.Bass) -> None:
    """This toolchain's walrus accepts only ONE sync-wait per instruction
    (setupSyncWait: 'Too many sync wait commands') while Tile emits several.
    Hoist all-but-one wait onto standalone EventSemaphore instructions
    placed immediately before, on the same engine — semantically identical
    (sequencer stalls at each wait in order)."""
    for f in nc.m.functions:
        for blk in f.blocks:
            insts = list(blk.instructions)
            out, changed = [], False
            for inst in insts:
                si = inst.sync_info
                if si is not None and len(si.on_wait) > 1:
                    waits = list(si.on_wait)
                    for i, w in enumerate(waits[:-1]):
                        ev = mybir.InstEventSemaphore(
                            name=f"{inst.name}-sw{i}", ins=[], outs=[]
                        )
                        ev.engine = inst.engine
                        ev.sync_info = mybir.SyncInfo(on_wait=[w], on_update=[])
                        out.append(ev)
                    si.on_wait = [waits[-1]]
                    changed = True
                out.append(inst)
            if changed:
                blk.instructions = out


def build_program(debug: bool = False, repeat: int = 1) -> bass.Bass:
    """Emit the single-core SPMD program (same program, per-core data).

    repeat > 1 re-runs the whole batch loop (idempotent) — used only for
    launch-overhead-cancelling timing in test.py.
    """
    nc = bass.Bass()
    if debug:
        d_dbg_mean = nc.declare_dram_parameter("dbg_mean", [B_LOC, 128, NCH], FP32, isOutput=True)
        d_dbg_h = nc.declare_dram_parameter("dbg_h", [B_LOC, MID, 1], FP32, isOutput=True)
        d_dbg_gat = nc.declare_dram_parameter("dbg_gat", [B_LOC, 128, NCH], FP32, isOutput=True)

    d_in = [
        nc.declare_dram_parameter(nm, [B_LOC, CH, 128, HW], FP32, isOutput=False)
        for nm in ("y", "x0", "x1", "x2", "x3")
    ]
    # Host-prepped params (see make_in_maps):
    #   w1t[p, j, m]  = conv1_w[m, 128j+p]
    #   w2t[m, j, p]  = conv2_w[128j+p, m]   (m<32); w2t[32, j, p] = conv2_b[128j+p]
    #                   columns j<CH (sigmoid) are pre-negated
    #   scale_eff/bias_eff: folded BN affine on the conv1 output
    d_w1t = nc.declare_dram_parameter("w1t", [128, NCH, MID], FP32, isOutput=False)
    d_w2t = nc.declare_dram_parameter("w2t", [MIDA, NCH, 128], FP32, isOutput=False)
    d_se = nc.declare_dram_parameter("scale_eff", [MID, 1], FP32, isOutput=False)
    d_be = nc.declare_dram_parameter("bias_eff", [MID, 1], FP32, isOutput=False)
    d_out = nc.declare_dram_parameter("out", [B_LOC, CH, 128, HW], FP32, isOutput=True)

    with tile.TileContext(nc) as tc, ExitStack() as ctx:
        cpool = ctx.enter_context(tc.tile_pool(name="cpool", bufs=1))
        ppool = ctx.enter_context(tc.tile_pool(name="ppool", bufs=2, space="PSUM"))
        dpool = ctx.enter_context(tc.tile_pool(name="dpool", bufs=2))
        spool = ctx.enter_context(tc.tile_pool(name="spool", bufs=2))

        # ---------------- parameter prep (once) ----------------
        # Contiguous loads of host-pretransposed weights, "laundered" through
        # one DVE copy each so PE matmuls (single sync-wait limit on their
        # embedded fp32 weight load) depend on a single producer proc (DVE).
        w1s = cpool.tile([128, NCH, MID], FP32, name="w1s", tag="w1s")
        w1T = cpool.tile([128, NCH, MID], FP32, name="w1T", tag="w1T")
        nc.sync.dma_start(out=w1s[:], in_=d_w1t[:])
        nc.vector.tensor_copy(w1T[:], w1s[:])

        w2s = cpool.tile([MIDA, NCH, 128], FP32, name="w2s", tag="w2s")
        w2T = cpool.tile([MIDA, NCH, 128], FP32, name="w2T", tag="w2T")
        nc.sync.dma_start(out=w2s[:], in_=d_w2t[:])
        nc.vector.tensor_copy(w2T[:], w2s[:])

        se_t = cpool.tile([MID, 1], FP32, name="se_t", tag="se_t")
        be_t = cpool.tile([MID, 1], FP32, name="be_t", tag="be_t")
        nc.sync.dma_start(out=se_t[:], in_=d_se[:])
        nc.sync.dma_start(out=be_t[:], in_=d_be[:])

        # ---------------- main loop over local batches ----------------
        for b in [i % B_LOC for i in range(B_LOC * repeat)]:
            # Load the 5 feature maps for this batch: [128, ch, hw].
            # Loads are spread over the SP and Pool DMA queues; neither
            # engine does compute, so their triggers run ahead freely.
            tiles = []
            load_eng = [nc.sync, nc.gpsimd, nc.sync, nc.sync, nc.gpsimd]
            for t in range(NT):
                dt_ = dpool.tile(
                    [128, CH, HW], FP32, name=f"d{t}", tag=f"d{t}", bufs=3
                )
                load_eng[t].dma_start(
                    out=dt_[:], in_=d_in[t][b].rearrange("c p f -> p c f")
                )
                tiles.append(dt_)

            # Channel sums -> mean_t[:, j], j = t*CH + ch.  All on DVE
            # (tensor_scalar accum, 2x fp32) so the PE matmul depends on a
            # single producer proc.
            mean_t = spool.tile([128, NCH], FP32, name="mean_t", tag="mean_t", bufs=2)
            scr = spool.tile([128, HW], FP32, name="scr", tag="scr", bufs=1)
            for t in range(NT):
                for ch in range(CH):
                    j = t * CH + ch
                    nc.vector.tensor_scalar(
                        out=scr[:],
                        in0=tiles[t][:, ch, :],
                        scalar1=1.0,
                        scalar2=None,
                        op0=ALU.mult,
                        op1=ALU.add,
                        accum_out=mean_t[:, j : j + 1],
                    )

            # Gate MLP on PE: h_raw[mid] = sum_j w1T[:,j,:].T @ sums[:,j]
            hps = ppool.tile([MID, 1], FP32, name="hps", tag="hps")
            for j in range(NCH):
                nc.tensor.matmul(
                    hps[:],
                    w1T[:, j, :],
                    mean_t[:, j : j + 1],
                    start=(j == 0),
                    stop=(j == NCH - 1),
                )
            # h = relu(h_raw*scale_eff + bias_eff); augmented with const 1.
            h33 = spool.tile([MIDA, 1], FP32, name="h33", tag="h33", bufs=2)
            nc.vector.memset(h33[MID : MID + 1, :], 1.0)
            nc.vector.tensor_scalar(
                out=h33[0:MID, :], in0=hps[:], scalar1=se_t[:], scalar2=be_t[:],
                op0=ALU.mult, op1=ALU.add,
            )
            nc.vector.tensor_scalar_max(out=h33[0:MID, :], in0=h33[0:MID, :], scalar1=0.0)

            # Logits, transposed into channel-on-partition layout, bias and
            # sigmoid sign pre-folded into w2T:
            #   wps[p, j<2]  = -(w[128j+p] )   wps[p, j>=2] = w[128j+p]
            wps = ppool.tile([128, NCH], FP32, name="wps", tag="wps")
            for j in range(NCH):
                nc.tensor.matmul(
                    wps[:, j : j + 1], w2T[:, j, :], h33[:], start=True, stop=True
                )

            # Single-function ACT: e = exp(wps) for all 10 columns.
            gatE = spool.tile([128, NCH], FP32, name="gatE", tag="gatE", bufs=2)
            nc.scalar.activation(out=gatE[:], in_=wps[:], func=AF.Exp)

            # Gates on DVE: sigmoid cols = 1/(1+e^-v); softmax cols = e/sum.
            gat = spool.tile([128, NCH], FP32, name="gat", tag="gat", bufs=2)
            t01 = spool.tile([128, CH], FP32, name="t01", tag="t01", bufs=2)
            nc.vector.tensor_scalar_add(out=t01[:], in0=gatE[:, 0:CH], scalar1=1.0)
            nc.vector.reciprocal(gat[:, 0:CH], t01[:])
            gkE = gatE[:, CH:NCH].rearrange("p (k c) -> p c k", c=CH)
            gk = gat[:, CH:NCH].rearrange("p (k c) -> p c k", c=CH)
            esum = spool.tile([128, CH, 1], FP32, name="esum", tag="esum", bufs=2)
            nc.vector.reduce_sum(out=esum[:], in_=gkE, axis=mybir.AxisListType.X)
            rinv = spool.tile([128, CH, 1], FP32, name="rinv", tag="rinv", bufs=2)
            nc.vector.reciprocal(rinv[:], esum[:])
            for ch in range(CH):
                nc.vector.tensor_scalar_mul(
                    out=gk[:, ch, :], in0=gkE[:, ch, :], scalar1=rinv[:, ch, :]
                )

            if debug:
                nc.sync.dma_start(out=d_dbg_mean[b], in_=mean_t[:])
                nc.sync.dma_start(out=d_dbg_h[b], in_=h33[0:MID, :])
                nc.sync.dma_start(out=d_dbg_gat[b], in_=gat[:])

            # Pass 2 on DVE: acc = y*w1 + sum_k x_k * g_k, then store (ACT q).
            acc = dpool.tile([128, CH, HW], FP32, name="acc", tag="acc", bufs=2)
            for ch in range(CH):
                nc.vector.tensor_scalar_mul(
                    out=acc[:, ch, :], in0=tiles[0][:, ch, :],
                    scalar1=gat[:, ch : ch + 1],
                )
                for k in range(K):
                    nc.vector.scalar_tensor_tensor(
                        out=acc[:, ch, :],
                        in0=tiles[1 + k][:, ch, :],
                        scalar=gat[:, CH + CH * k + ch : CH + CH * k + ch + 1],
                        in1=acc[:, ch, :],
                        op0=ALU.mult,
                        op1=ALU.add,
                    )
            nc.scalar.dma_start(out=d_out[b].rearrange("c p f -> p c f"), in_=acc[:])

    _split_waits(nc)
    return nc


_CACHE: dict = {}


def _get_program() -> bass.Bass:
    if "nc" not in _CACHE:
        _CACHE["nc"] = build_program()
    return _CACHE["nc"]


def make_in_maps(inputs: dict) -> list:
    """Shard full inputs into per-core input maps (batch-parallel) and
    pre-transpose/fold the tiny params on the host."""
    f32 = lambda a: np.ascontiguousarray(np.asarray(a), dtype=np.float32)
    y = f32(inputs["y"]).reshape(B, CH, 128, HW)
    xs = [f32(inputs[f"x{k}"]).reshape(B, CH, 128, HW) for k in range(K)]

    c1 = f32(inputs["conv1_w"])                       # [MID, FEAT]
    w1t = np.ascontiguousarray(
        c1.reshape(MID, NCH, 128).transpose(2, 1, 0)  # [p, j, m]
    )
    c2 = f32(inputs["conv2_w"])                       # [FEAT, MID]
    c2b = f32(inputs["conv2_b"])                      # [FEAT]
    w2t = np.empty((MIDA, NCH, 128), np.float32)
    w2t[:MID] = c2.reshape(NCH, 128, MID).transpose(2, 0, 1)  # [m, j, p]
    w2t[MID] = c2b.reshape(NCH, 128)
    w2t[:, :CH, :] *= -1.0                            # sigmoid cols: exp(-v)

    g = f32(inputs["bn_gamma"])
    bta = f32(inputs["bn_beta"])
    mu = f32(inputs["bn_mean"])
    var = f32(inputs["bn_var"])
    s = g / np.sqrt(var + EPS)
    scale_eff = np.ascontiguousarray((s / HW).reshape(MID, 1))
    bias_eff = np.ascontiguousarray((bta - mu * s).reshape(MID, 1))

    shared = {
        "w1t": w1t,
        "w2t": np.ascontiguousarray(w2t),
        "scale_eff": scale_eff,
        "bias_eff": bias_eff,
    }
    in_maps = []
    for core in range(N_CORES):
        sl = slice(core * B_LOC, (core + 1) * B_LOC)
        m = {"y": np.ascontiguousarray(y[sl])}
        for k in range(K):
            m[f"x{k}"] = np.ascontiguousarray(xs[k][sl])
        m.update(shared)
        in_maps.append(m)
    return in_maps


def kernel(**inputs) -> np.ndarray:
    nc = _get_program()
    in_maps = make_in_maps(inputs)
    res = run_bass_kernel_spmd(nc, in_maps, list(range(N_CORES)))
    _CACHE["last_results"] = res
    out = np.concatenate(
        [res.results[i]["out"].reshape(B_LOC, C, H, W) for i in range(N_CORES)],
        axis=0,
    )
    return out.astype(np.float32)


# revision 3
# speedup vs baseline: 1.0222x; 1.0222x over previous
"""Trainium2 Bass kernel for nn_GatedJunction (gated multi-branch junction).

Math (per batch element b):
    m_y  = mean_hw(y[b])                     # [C]
    m_xk = mean_hw(x_k[b])                   # [C] for k=0..3
    feats = concat(m_y, m_x0..m_x3)          # [5C] = [1280]
    h  = relu(bn(feats @ conv1_w.T))         # [32]
    w  = h @ conv2_w.T + conv2_b             # [1280] -> [5, 256]
    w1 = sigmoid(w[0])                       # self gate  [256]
    w2 = softmax_k(w[1:])                    # branch gates [4, 256]
    out[b] = y[b]*w1[:,None,None] + sum_k w2[k][:,None,None]*x_k[b]

Sharding: data-parallel over batch. 8 cores x 4 batch elements each.
Params are tiny; they are pre-transposed/folded on the host and replicated.

v2 design notes (vs the v1 baseline):
- Host-side param prep: conv1_w/conv2_w pre-transposed into the on-chip
  layouts, BN folded to (scale_eff, bias_eff), conv2_b folded into an
  augmented 33rd weight row, and the sigmoid columns pre-negated so that
  ALL ten gate columns need only a single Exp activation.  The ACT engine
  therefore runs exactly one function (Exp) -> no activation-table reloads.
- All other elementwise work (means via tensor_scalar accum, BN affine,
  relu via max, sigmoid reconstruction 1/(1+e^-v), softmax normalization,
  pass-2 FMAs) runs on DVE (2x fp32 mode).
- DMA is spread over independent engine queues: loads of y/x1/x2 on the
  SP queue, x0/x3 on the Pool queue, output stores on the ACT queue, so
  transfers pipeline and per-DMA gaps overlap across queues.
"""

import sys

for _p in ("/root/.axon_site/_ro/trn_rl_repo", "/opt/trn_rl_repo"):
    if _p not in sys.path:
        sys.path.append(_p)

from contextlib import ExitStack

import numpy as np

import concourse.bass as bass
import concourse.tile as tile
from concourse import mybir
from concourse.bass_utils import run_bass_kernel_spmd

# Problem constants (hardcoded from the spec).
B, K, C, H, W = 32, 4, 256, 32, 32
MID = 32
EPS = 1e-5
HW = H * W          # 1024
N_CORES = 8
B_LOC = B // N_CORES  # 4
NT = K + 1          # 5 tensors: y, x0..x3
FEAT = NT * C       # 1280
NCH = FEAT // 128   # 10 feature chunks of 128
CH = C // 128       # 2 channel chunks per tensor
MIDA = MID + 1      # 33: h augmented with a constant-1 row (bias fold)

FP32 = mybir.dt.float32
ALU = mybir.AluOpType
AF = mybir.ActivationFunctionType


def _split_waits(nc: bass.Bass) -> None:
    """This toolchain's walrus accepts only ONE sync-wait per instruction
    (setupSyncWait: 'Too many sync wait commands') while Tile emits several.
    Hoist all-but-one wait onto standalone EventSemaphore instructions
    placed immediately before, on the same engine — semantically identical
    (sequencer stalls at each wait in order)."""
    for f in nc.m.functions:
        for blk in f.blocks:
            insts = list(blk.instructions)
            out, changed = [], False
            for inst in insts:
                si = inst.sync_info
                if si is not None and len(si.on_wait) > 1:
                    waits = list(si.on_wait)
                    for i, w in enumerate(waits[:-1]):
                        ev = mybir.InstEventSemaphore(
                            name=f"{inst.name}-sw{i}", ins=[], outs=[]
                        )
                        ev.engine = inst.engine
                        ev.sync_info = mybir.SyncInfo(on_wait=[w], on_update=[])
                        out.append(ev)
                    si.on_wait = [waits[-1]]
                    changed = True
                out.append(inst)
            if changed:
                blk.instructions = out


def build_program(debug: bool = False, repeat: int = 1) -> bass.Bass:
    """Emit the single-core SPMD program (same program, per-core data).

    repeat > 1 re-runs the whole batch loop (idempotent) — used only for
    launch-overhead-cancelling timing in test.py.
    """
    nc = bass.Bass()
    if debug:
        d_dbg_mean = nc.declare_dram_parameter("dbg_mean", [B_LOC, 128, NCH], FP32, isOutput=True)
        d_dbg_h = nc.declare_dram_parameter("dbg_h", [B_LOC, MID, 1], FP32, isOutput=True)
        d_dbg_gat = nc.declare_dram_parameter("dbg_gat", [B_LOC, 128, NCH], FP32, isOutput=True)

    d_in = [
        nc.declare_dram_parameter(nm, [B_LOC, CH, 128, HW], FP32, isOutput=False)
        for nm in ("y", "x0", "x1", "x2", "x3")
    ]
    # Host-prepped params (see make_in_maps):
    #   w1t[p, j, m]  = conv1_w[m, 128j+p]
    #   w2t[m, j, p]  = conv2_w[128j+p, m]   (m<32); w2t[32, j, p] = conv2_b[128j+p]
    #                   columns j<CH (sigmoid) are pre-negated
    #   scale_eff/bias_eff: folded BN affine on the conv1 output
    d_w1t = nc.declare_dram_parameter("w1t", [128, NCH, MID], FP32, isOutput=False)
    d_w2t = nc.declare_dram_parameter("w2t", [MIDA, NCH, 128], FP32, isOutput=False)
    d_se = nc.declare_dram_parameter("scale_eff", [MID, 1], FP32, isOutput=False)
    d_be = nc.declare_dram_parameter("bias_eff", [MID, 1], FP32, isOutput=False)
    d_out = nc.declare_dram_parameter("out", [B_LOC, CH, 128, HW], FP32, isOutput=True)

    with tile.TileContext(nc) as tc, ExitStack() as ctx:
        cpool = ctx.enter_context(tc.tile_pool(name="cpool", bufs=1))
        ppool = ctx.enter_context(tc.tile_pool(name="ppool", bufs=2, space="PSUM"))
        dpool = ctx.enter_context(tc.tile_pool(name="dpool", bufs=2))
        spool = ctx.enter_context(tc.tile_pool(name="spool", bufs=2))

        # ---------------- parameter prep (once) ----------------
        # Contiguous loads of host-pretransposed weights, "laundered" through
        # one DVE copy each so PE matmuls (single sync-wait limit on their
        # embedded fp32 weight load) depend on a single producer proc (DVE).
        w1s = cpool.tile([128, NCH, MID], FP32, name="w1s", tag="w1s")
        w1T = cpool.tile([128, NCH, MID], FP32, name="w1T", tag="w1T")
        nc.sync.dma_start(out=w1s[:], in_=d_w1t[:])
        nc.vector.tensor_copy(w1T[:], w1s[:])

        w2s = cpool.tile([MIDA, NCH, 128], FP32, name="w2s", tag="w2s")
        w2T = cpool.tile([MIDA, NCH, 128], FP32, name="w2T", tag="w2T")
        nc.sync.dma_start(out=w2s[:], in_=d_w2t[:])
        nc.vector.tensor_copy(w2T[:], w2s[:])

        se_t = cpool.tile([MID, 1], FP32, name="se_t", tag="se_t")
        be_t = cpool.tile([MID, 1], FP32, name="be_t", tag="be_t")
        nc.sync.dma_start(out=se_t[:], in_=d_se[:])
        nc.sync.dma_start(out=be_t[:], in_=d_be[:])

        # ---------------- main loop over local batches ----------------
        for b in [i % B_LOC for i in range(B_LOC * repeat)]:
            # Load the 5 feature maps for this batch: [128, ch, hw].
            # Loads are spread over the SP and Pool DMA queues; neither
            # engine does compute, so their triggers run ahead freely.
            tiles = []
            load_eng = [nc.sync, nc.gpsimd, nc.sync, nc.sync, nc.gpsimd]
            for t in range(NT):
                dt_ = dpool.tile(
                    [128, CH, HW], FP32, name=f"d{t}", tag=f"d{t}", bufs=3
                )
                load_eng[t].dma_start(
                    out=dt_[:], in_=d_in[t][b].rearrange("c p f -> p c f")
                )
                tiles.append(dt_)

            # Channel sums -> mean_t[:, j], j = t*CH + ch.  All on DVE
            # (tensor_scalar accum, 2x fp32) so the PE matmul depends on a
            # single producer proc.
            mean_t = spool.tile([128, NCH], FP32, name="mean_t", tag="mean_t", bufs=2)
            scr = spool.tile([128, HW], FP32, name="scr", tag="scr", bufs=1)
            for t in range(NT):
                for ch in range(CH):
                    j = t * CH + ch
                    nc.vector.tensor_scalar(
                        out=scr[:],
                        in0=tiles[t][:, ch, :],
                        scalar1=1.0,
                        scalar2=None,
                        op0=ALU.mult,
                        op1=ALU.add,
                        accum_out=mean_t[:, j : j + 1],
                    )

            # Gate MLP on PE: h_raw[mid] = sum_j w1T[:,j,:].T @ sums[:,j]
            hps = ppool.tile([MID, 1], FP32, name="hps", tag="hps")
            for j in range(NCH):
                nc.tensor.matmul(
                    hps[:],
                    w1T[:, j, :],
                    mean_t[:, j : j + 1],
                    start=(j == 0),
                    stop=(j == NCH - 1),
                )
            # h = relu(h_raw*scale_eff + bias_eff); augmented with const 1.
            h33 = spool.tile([MIDA, 1], FP32, name="h33", tag="h33", bufs=2)
            nc.vector.memset(h33[MID : MID + 1, :], 1.0)
            nc.vector.tensor_scalar(
                out=h33[0:MID, :], in0=hps[:], scalar1=se_t[:], scalar2=be_t[:],
                op0=ALU.mult, op1=ALU.add,
            )
            nc.vector.tensor_scalar_max(out=h33[0:MID, :], in0=h33[0:MID, :], scalar1=0.0)

            # Logits, transposed into channel-on-partition layout, bias and
            # sigmoid sign pre-folded into w2T:
            #   wps[p, j<2]  = -(w[128j+p] )   wps[p, j>=2] = w[128j+p]
            wps = ppool.tile([128, NCH], FP32, name="wps", tag="wps")
            for j in range(NCH):
                nc.tensor.matmul(
                    wps[:, j : j + 1], w2T[:, j, :], h33[:], start=True, stop=True
                )

            # Single-function ACT: e = exp(wps) for all 10 columns.
            gatE = spool.tile([128, NCH], FP32, name="gatE", tag="gatE", bufs=2)
            nc.scalar.activation(out=gatE[:], in_=wps[:], func=AF.Exp)

            # Gates on DVE: sigmoid cols = 1/(1+e^-v); softmax cols = e/sum.
            gat = spool.tile([128, NCH], FP32, name="gat", tag="gat", bufs=2)
            t01 = spool.tile([128, CH], FP32, name="t01", tag="t01", bufs=2)
            nc.vector.tensor_scalar_add(out=t01[:], in0=gatE[:, 0:CH], scalar1=1.0)
            nc.vector.reciprocal(gat[:, 0:CH], t01[:])
            gkE = gatE[:, CH:NCH].rearrange("p (k c) -> p c k", c=CH)
            gk = gat[:, CH:NCH].rearrange("p (k c) -> p c k", c=CH)
            esum = spool.tile([128, CH, 1], FP32, name="esum", tag="esum", bufs=2)
            nc.vector.reduce_sum(out=esum[:], in_=gkE, axis=mybir.AxisListType.X)
            rinv = spool.tile([128, CH, 1], FP32, name="rinv", tag="rinv", bufs=2)
            nc.vector.reciprocal(rinv[:], esum[:])
            for ch in range(CH):
                nc.vector.tensor_scalar_mul(
                    out=gk[:, ch, :], in0=gkE[:, ch, :], scalar1=rinv[:, ch, :]
                )

            if debug:
                nc.sync.dma_start(out=d_dbg_mean[b], in_=mean_t[:])
                nc.sync.dma_start(out=d_dbg_h[b], in_=h33[0:MID, :])
                nc.sync.dma_start(out=d_dbg_gat[b], in_=gat[:])

            # Pass 2 on DVE: acc = y*w1 + sum_k x_k * g_k; store each channel
            # chunk as soon as it completes (ACT queue) to shrink the tail.
            acc = dpool.tile([128, CH, HW], FP32, name="acc", tag="acc", bufs=2)
            for ch in range(CH):
                nc.vector.tensor_scalar_mul(
                    out=acc[:, ch, :], in0=tiles[0][:, ch, :],
                    scalar1=gat[:, ch : ch + 1],
                )
                for k in range(K):
                    nc.vector.scalar_tensor_tensor(
                        out=acc[:, ch, :],
                        in0=tiles[1 + k][:, ch, :],
                        scalar=gat[:, CH + CH * k + ch : CH + CH * k + ch + 1],
                        in1=acc[:, ch, :],
                        op0=ALU.mult,
                        op1=ALU.add,
                    )
                nc.scalar.dma_start(out=d_out[b][ch], in_=acc[:, ch, :])

    _split_waits(nc)
    return nc


_CACHE: dict = {}


def _get_program() -> bass.Bass:
    if "nc" not in _CACHE:
        _CACHE["nc"] = build_program()
    return _CACHE["nc"]


def make_in_maps(inputs: dict) -> list:
    """Shard full inputs into per-core input maps (batch-parallel) and
    pre-transpose/fold the tiny params on the host."""
    f32 = lambda a: np.ascontiguousarray(np.asarray(a), dtype=np.float32)
    y = f32(inputs["y"]).reshape(B, CH, 128, HW)
    xs = [f32(inputs[f"x{k}"]).reshape(B, CH, 128, HW) for k in range(K)]

    c1 = f32(inputs["conv1_w"])                       # [MID, FEAT]
    w1t = np.ascontiguousarray(
        c1.reshape(MID, NCH, 128).transpose(2, 1, 0)  # [p, j, m]
    )
    c2 = f32(inputs["conv2_w"])                       # [FEAT, MID]
    c2b = f32(inputs["conv2_b"])                      # [FEAT]
    w2t = np.empty((MIDA, NCH, 128), np.float32)
    w2t[:MID] = c2.reshape(NCH, 128, MID).transpose(2, 0, 1)  # [m, j, p]
    w2t[MID] = c2b.reshape(NCH, 128)
    w2t[:, :CH, :] *= -1.0                            # sigmoid cols: exp(-v)

    g = f32(inputs["bn_gamma"])
    bta = f32(inputs["bn_beta"])
    mu = f32(inputs["bn_mean"])
    var = f32(inputs["bn_var"])
    s = g / np.sqrt(var + EPS)
    scale_eff = np.ascontiguousarray((s / HW).reshape(MID, 1))
    bias_eff = np.ascontiguousarray((bta - mu * s).reshape(MID, 1))

    shared = {
        "w1t": w1t,
        "w2t": np.ascontiguousarray(w2t),
        "scale_eff": scale_eff,
        "bias_eff": bias_eff,
    }
    in_maps = []
    for core in range(N_CORES):
        sl = slice(core * B_LOC, (core + 1) * B_LOC)
        m = {"y": np.ascontiguousarray(y[sl])}
        for k in range(K):
            m[f"x{k}"] = np.ascontiguousarray(xs[k][sl])
        m.update(shared)
        in_maps.append(m)
    return in_maps


def kernel(**inputs) -> np.ndarray:
    nc = _get_program()
    in_maps = make_in_maps(inputs)
    res = run_bass_kernel_spmd(nc, in_maps, list(range(N_CORES)))
    _CACHE["last_results"] = res
    out = np.concatenate(
        [res.results[i]["out"].reshape(B_LOC, C, H, W) for i in range(N_CORES)],
        axis=0,
    )
    return out.astype(np.float32)


# revision 5
# speedup vs baseline: 1.1015x; 1.0775x over previous
"""Trainium2 Bass kernel for nn_GatedJunction (gated multi-branch junction).

Math (per batch element b):
    m_y  = mean_hw(y[b])                     # [C]
    m_xk = mean_hw(x_k[b])                   # [C] for k=0..3
    feats = concat(m_y, m_x0..m_x3)          # [5C] = [1280]
    h  = relu(bn(feats @ conv1_w.T))         # [32]
    w  = h @ conv2_w.T + conv2_b             # [1280] -> [5, 256]
    w1 = sigmoid(w[0])                       # self gate  [256]
    w2 = softmax_k(w[1:])                    # branch gates [4, 256]
    out[b] = y[b]*w1[:,None,None] + sum_k w2[k][:,None,None]*x_k[b]

Sharding: data-parallel over batch. 8 cores x 4 batch elements each.
Params are tiny; they are pre-transposed/folded on the host and replicated.

Design notes (vs the v1 baseline):
- Host-side param prep: conv1_w/conv2_w pre-transposed into the on-chip
  layouts, BN folded to (scale_eff, bias_eff), conv2_b folded into an
  augmented 33rd weight row, and the sigmoid columns pre-negated so that
  ALL ten gate columns need only a single Exp activation.
- Compute is balanced across ACT and DVE so neither backlogs behind the
  DMA stream: y/x0 channel sums and the y*w1 base term run on ACT
  (Copy + accum_out / per-partition scale), x1..x3 sums, BN affine, relu
  via max, sigmoid reconstruction 1/(1+e^-v), softmax normalization and
  the four pass-2 FMAs run on DVE (2x fp32 mode).
- DMA is spread over independent engine queues: loads of y/x1/x2 on the
  SP queue, x0/x3 on the Pool (SWDGE) queue; per-channel output stores
  issue from SP as soon as each half of acc completes, shrinking the
  single-shot tail.
"""

import sys

for _p in ("/root/.axon_site/_ro/trn_rl_repo", "/opt/trn_rl_repo"):
    if _p not in sys.path:
        sys.path.append(_p)

from contextlib import ExitStack

import numpy as np

import concourse.bass as bass
import concourse.tile as tile
from concourse import mybir
from concourse.bass_utils import run_bass_kernel_spmd

# Problem constants (hardcoded from the spec).
B, K, C, H, W = 32, 4, 256, 32, 32
MID = 32
EPS = 1e-5
HW = H * W          # 1024
N_CORES = 8
B_LOC = B // N_CORES  # 4
NT = K + 1          # 5 tensors: y, x0..x3
FEAT = NT * C       # 1280
NCH = FEAT // 128   # 10 feature chunks of 128
CH = C // 128       # 2 channel chunks per tensor
MIDA = MID + 1      # 33: h augmented with a constant-1 row (bias fold)

FP32 = mybir.dt.float32
ALU = mybir.AluOpType
AF = mybir.ActivationFunctionType


def _split_waits(nc: bass.Bass) -> None:
    """This toolchain's walrus accepts only ONE sync-wait per instruction
    (setupSyncWait: 'Too many sync wait commands') while Tile emits several.
    Hoist all-but-one wait onto standalone EventSemaphore instructions
    placed immediately before, on the same engine — semantically identical
    (sequencer stalls at each wait in order)."""
    for f in nc.m.functions:
        for blk in f.blocks:
            insts = list(blk.instructions)
            out, changed = [], False
            for inst in insts:
                si = inst.sync_info
                if si is not None and len(si.on_wait) > 1:
                    waits = list(si.on_wait)
                    for i, w in enumerate(waits[:-1]):
                        ev = mybir.InstEventSemaphore(
                            name=f"{inst.name}-sw{i}", ins=[], outs=[]
                        )
                        ev.engine = inst.engine
                        ev.sync_info = mybir.SyncInfo(on_wait=[w], on_update=[])
                        out.append(ev)
                    si.on_wait = [waits[-1]]
                    changed = True
                out.append(inst)
            if changed:
                blk.instructions = out


def build_program(debug: bool = False, repeat: int = 1) -> bass.Bass:
    """Emit the single-core SPMD program (same program, per-core data).

    repeat > 1 re-runs the whole batch loop (idempotent) — used only for
    launch-overhead-cancelling timing in test.py.
    """
    nc = bass.Bass()
    if debug:
        d_dbg_mean = nc.declare_dram_parameter("dbg_mean", [B_LOC, 128, NCH], FP32, isOutput=True)
        d_dbg_h = nc.declare_dram_parameter("dbg_h", [B_LOC, MID, 1], FP32, isOutput=True)
        d_dbg_gat = nc.declare_dram_parameter("dbg_gat", [B_LOC, 128, NCH], FP32, isOutput=True)

    d_in = [
        nc.declare_dram_parameter(nm, [B_LOC, CH, 128, HW], FP32, isOutput=False)
        for nm in ("y", "x0", "x1", "x2", "x3")
    ]
    # Host-prepped params (see make_in_maps):
    #   w1t[p, j, m]  = conv1_w[m, 128j+p]
    #   w2t[m, j, p]  = conv2_w[128j+p, m]   (m<32); w2t[32, j, p] = conv2_b[128j+p]
    #                   columns j<CH (sigmoid) are pre-negated
    #   scale_eff/bias_eff: folded BN affine on the conv1 output
    d_w1t = nc.declare_dram_parameter("w1t", [128, NCH, MID], FP32, isOutput=False)
    d_w2t = nc.declare_dram_parameter("w2t", [MIDA, NCH, 128], FP32, isOutput=False)
    d_se = nc.declare_dram_parameter("scale_eff", [MID, 1], FP32, isOutput=False)
    d_be = nc.declare_dram_parameter("bias_eff", [MID, 1], FP32, isOutput=False)
    d_out = nc.declare_dram_parameter("out", [B_LOC, CH, 128, HW], FP32, isOutput=True)

    with tile.TileContext(nc) as tc, ExitStack() as ctx:
        cpool = ctx.enter_context(tc.tile_pool(name="cpool", bufs=1))
        ppool = ctx.enter_context(tc.tile_pool(name="ppool", bufs=2, space="PSUM"))
        dpool = ctx.enter_context(tc.tile_pool(name="dpool", bufs=2))
        spool = ctx.enter_context(tc.tile_pool(name="spool", bufs=2))

        # ---------------- parameter prep (once) ----------------
        # Contiguous loads of host-pretransposed weights, "laundered" through
        # one DVE copy each so PE matmuls (single sync-wait limit on their
        # embedded fp32 weight load) depend on a single producer proc (DVE).
        w1s = cpool.tile([128, NCH, MID], FP32, name="w1s", tag="w1s")
        w1T = cpool.tile([128, NCH, MID], FP32, name="w1T", tag="w1T")
        nc.sync.dma_start(out=w1s[:], in_=d_w1t[:])
        nc.vector.tensor_copy(w1T[:], w1s[:])

        w2s = cpool.tile([MIDA, NCH, 128], FP32, name="w2s", tag="w2s")
        w2T = cpool.tile([MIDA, NCH, 128], FP32, name="w2T", tag="w2T")
        nc.sync.dma_start(out=w2s[:], in_=d_w2t[:])
        nc.vector.tensor_copy(w2T[:], w2s[:])

        se_t = cpool.tile([MID, 1], FP32, name="se_t", tag="se_t")
        be_t = cpool.tile([MID, 1], FP32, name="be_t", tag="be_t")
        nc.sync.dma_start(out=se_t[:], in_=d_se[:])
        nc.sync.dma_start(out=be_t[:], in_=d_be[:])

        # ---------------- main loop over local batches ----------------
        pending_store = None  # (b, acc) deferred store triggers (ACT queue)
        for b in [i % B_LOC for i in range(B_LOC * repeat)]:
            # Load the 5 feature maps for this batch: [128, ch, hw].
            # Loads are spread over the SP and Pool DMA queues; neither
            # engine does compute, so their triggers run ahead freely.
            tiles = []
            load_eng = [nc.sync, nc.gpsimd, nc.sync, nc.sync, nc.gpsimd]
            for t in range(NT):
                dt_ = dpool.tile(
                    [128, CH, HW], FP32, name=f"d{t}", tag=f"d{t}", bufs=3
                )
                load_eng[t].dma_start(
                    out=dt_[:], in_=d_in[t][b].rearrange("c p f -> p c f")
                )
                tiles.append(dt_)

            # Channel sums -> mean_t[:, j], j = t*CH + ch.  Split ACT/DVE:
            # y and x0 sums on ACT (Copy + accum_out), x1..x3 on DVE
            # (tensor_scalar accum, 2x fp32) — balances the two engines so
            # the DVE backlog doesn't trail the DMA stream at the tail.
            mean_t = spool.tile([128, NCH], FP32, name="mean_t", tag="mean_t", bufs=2)
            scr = spool.tile([128, HW], FP32, name="scr", tag="scr", bufs=1)
            for t in range(2):
                for ch in range(CH):
                    scr_a = spool.tile([128, HW], FP32, name="scr_a", tag="scr_a", bufs=2)
                    nc.scalar.activation(
                        out=scr_a[:],
                        in_=tiles[t][:, ch, :],
                        func=AF.Copy,
                        accum_out=mean_t[:, t * CH + ch : t * CH + ch + 1],
                    )
            for t in range(2, NT):
                for ch in range(CH):
                    j = t * CH + ch
                    nc.vector.tensor_scalar(
                        out=scr[:],
                        in0=tiles[t][:, ch, :],
                        scalar1=1.0,
                        scalar2=None,
                        op0=ALU.mult,
                        op1=ALU.add,
                        accum_out=mean_t[:, j : j + 1],
                    )

            # Gate MLP on PE: h_raw[mid] = sum_j w1T[:,j,:].T @ sums[:,j]
            hps = ppool.tile([MID, 1], FP32, name="hps", tag="hps")
            for j in range(NCH):
                nc.tensor.matmul(
                    hps[:],
                    w1T[:, j, :],
                    mean_t[:, j : j + 1],
                    start=(j == 0),
                    stop=(j == NCH - 1),
                )
            # h = relu(h_raw*scale_eff + bias_eff); augmented with const 1.
            h33 = spool.tile([MIDA, 1], FP32, name="h33", tag="h33", bufs=2)
            nc.vector.memset(h33[MID : MID + 1, :], 1.0)
            nc.vector.tensor_scalar(
                out=h33[0:MID, :], in0=hps[:], scalar1=se_t[:], scalar2=be_t[:],
                op0=ALU.mult, op1=ALU.add,
            )
            nc.vector.tensor_scalar_max(out=h33[0:MID, :], in0=h33[0:MID, :], scalar1=0.0)

            # Logits, transposed into channel-on-partition layout, bias and
            # sigmoid sign pre-folded into w2T:
            #   wps[p, j<2]  = -(w[128j+p] )   wps[p, j>=2] = w[128j+p]
            wps = ppool.tile([128, NCH], FP32, name="wps", tag="wps")
            for j in range(NCH):
                nc.tensor.matmul(
                    wps[:, j : j + 1], w2T[:, j, :], h33[:], start=True, stop=True
                )

            # Single-function ACT: e = exp(wps) for all 10 columns.
            gatE = spool.tile([128, NCH], FP32, name="gatE", tag="gatE", bufs=2)
            nc.scalar.activation(out=gatE[:], in_=wps[:], func=AF.Exp)

            # Gates on DVE: sigmoid cols = 1/(1+e^-v); softmax cols = e/sum.
            gat = spool.tile([128, NCH], FP32, name="gat", tag="gat", bufs=2)
            t01 = spool.tile([128, CH], FP32, name="t01", tag="t01", bufs=2)
            nc.vector.tensor_scalar_add(out=t01[:], in0=gatE[:, 0:CH], scalar1=1.0)
            nc.vector.reciprocal(gat[:, 0:CH], t01[:])
            gkE = gatE[:, CH:NCH].rearrange("p (k c) -> p c k", c=CH)
            gk = gat[:, CH:NCH].rearrange("p (k c) -> p c k", c=CH)
            esum = spool.tile([128, CH, 1], FP32, name="esum", tag="esum", bufs=2)
            nc.vector.reduce_sum(out=esum[:], in_=gkE, axis=mybir.AxisListType.X)
            rinv = spool.tile([128, CH, 1], FP32, name="rinv", tag="rinv", bufs=2)
            nc.vector.reciprocal(rinv[:], esum[:])
            for ch in range(CH):
                nc.vector.tensor_scalar_mul(
                    out=gk[:, ch, :], in0=gkE[:, ch, :], scalar1=rinv[:, ch, :]
                )

            if debug:
                nc.sync.dma_start(out=d_dbg_mean[b], in_=mean_t[:])
                nc.sync.dma_start(out=d_dbg_h[b], in_=h33[0:MID, :])
                nc.sync.dma_start(out=d_dbg_gat[b], in_=gat[:])

            # Pass 2: y*w1 base term on ACT (Copy with per-partition scale),
            # the four FMAs on DVE.  Stores are deferred to the next batch.
            acc = dpool.tile([128, CH, HW], FP32, name="acc", tag="acc", bufs=2)
            for ch in range(CH):
                nc.scalar.activation(
                    out=acc[:, ch, :], in_=tiles[0][:, ch, :],
                    func=AF.Copy, scale=gat[:, ch : ch + 1],
                )
                for k in range(K):
                    nc.vector.scalar_tensor_tensor(
                        out=acc[:, ch, :],
                        in0=tiles[1 + k][:, ch, :],
                        scalar=gat[:, CH + CH * k + ch : CH + CH * k + ch + 1],
                        in1=acc[:, ch, :],
                        op0=ALU.mult,
                        op1=ALU.add,
                    )
                nc.sync.dma_start(out=d_out[b][ch], in_=acc[:, ch, :])

    _split_waits(nc)
    return nc


_CACHE: dict = {}


def _get_program() -> bass.Bass:
    if "nc" not in _CACHE:
        _CACHE["nc"] = build_program()
    return _CACHE["nc"]


def make_in_maps(inputs: dict) -> list:
    """Shard full inputs into per-core input maps (batch-parallel) and
    pre-transpose/fold the tiny params on the host."""
    f32 = lambda a: np.ascontiguousarray(np.asarray(a), dtype=np.float32)
    y = f32(inputs["y"]).reshape(B, CH, 128, HW)
    xs = [f32(inputs[f"x{k}"]).reshape(B, CH, 128, HW) for k in range(K)]

    c1 = f32(inputs["conv1_w"])                       # [MID, FEAT]
    w1t = np.ascontiguousarray(
        c1.reshape(MID, NCH, 128).transpose(2, 1, 0)  # [p, j, m]
    )
    c2 = f32(inputs["conv2_w"])                       # [FEAT, MID]
    c2b = f32(inputs["conv2_b"])                      # [FEAT]
    w2t = np.empty((MIDA, NCH, 128), np.float32)
    w2t[:MID] = c2.reshape(NCH, 128, MID).transpose(2, 0, 1)  # [m, j, p]
    w2t[MID] = c2b.reshape(NCH, 128)
    w2t[:, :CH, :] *= -1.0                            # sigmoid cols: exp(-v)

    g = f32(inputs["bn_gamma"])
    bta = f32(inputs["bn_beta"])
    mu = f32(inputs["bn_mean"])
    var = f32(inputs["bn_var"])
    s = g / np.sqrt(var + EPS)
    scale_eff = np.ascontiguousarray((s / HW).reshape(MID, 1))
    bias_eff = np.ascontiguousarray((bta - mu * s).reshape(MID, 1))

    shared = {
        "w1t": w1t,
        "w2t": np.ascontiguousarray(w2t),
        "scale_eff": scale_eff,
        "bias_eff": bias_eff,
    }
    in_maps = []
    for core in range(N_CORES):
        sl = slice(core * B_LOC, (core + 1) * B_LOC)
        m = {"y": np.ascontiguousarray(y[sl])}
        for k in range(K):
            m[f"x{k}"] = np.ascontiguousarray(xs[k][sl])
        m.update(shared)
        in_maps.append(m)
    return in_maps


def kernel(**inputs) -> np.ndarray:
    nc = _get_program()
    in_maps = make_in_maps(inputs)
    res = run_bass_kernel_spmd(nc, in_maps, list(range(N_CORES)))
    _CACHE["last_results"] = res
    out = np.concatenate(
        [res.results[i]["out"].reshape(B_LOC, C, H, W) for i in range(N_CORES)],
        axis=0,
    )
    return out.astype(np.float32)
